# revision 1
# baseline (speedup 1.0000x reference)
"""Elman RNN on 8 Trainium2 NeuronCores.

Strategy: time-shard T=512 across the 8 cores (64 owned steps each) and
exploit the contractivity of the relu recurrence: each core re-runs a
48-step burn-in from h=0 before its owned window, which converges to the
true hidden state to ~5e-7 relative error (fp32 noise floor); the first
24 burn-in steps feed bf16 x (their rounding error also contracts away).
Core 0 has no real predecessor steps; its burn-in input is a forcing
vector x* with W_x @ x* = -1e4, so relu clamps h to exactly 0 until its
window starts.

On-chip layout is transposed: the hidden state g = h^T lives as
(D=128 partitions, N=256 free). Per step:
  PE:   psum[:, step] += W_h^T.T @ g_prev      (xproj pre-filled per pair)
  ACT:  gA = relu(psum[:, nA] + b_x)           (batch half A)
  DVE:  gB = relu(psum[:, nB] + b_x)           (batch half B)
Owned steps: y^T = W_y^T.T @ g into PSUM (evacuated per 4-step quad on
DVE with b_y added as a per-partition bias), h^T DMA'd straight from the
g tiles. Both outputs are written transposed — (K, OWN*N) / (D, OWN*N) —
and the host untransposes during reassembly. This keeps the PE free of
transpose and bias matmuls (fp32 matmul/LDWEIGHTS are 2-pass on trn2,
so every avoided PE op counts double).
"""

import sys

if "/opt/trn_rl_repo" not in sys.path:
    sys.path.insert(0, "/opt/trn_rl_repo")

import numpy as np

T, N, C, D, K = 512, 256, 128, 128, 128
NCORES = 8
OWN = T // NCORES          # 64 owned timesteps per core
BURN = 48                  # burn-in steps (contraction reaches fp32 floor)
NBF = 24                   # leading burn-in steps fed bf16 x (errors contract)
S = OWN + BURN             # 112 recurrence steps per core
FORCE = 1.0e4
HALF = N // 2              # 128: batch half per relu chain
PF = 2                     # xproj prefetch depth, in pairs
BF_PAIRS = NBF // 2        # pairs taking the bf16 xproj path
OQ = OWN // 4              # owned quads (4-step output groups)

_prog_cache = {}


def _build_program(repeats=1, bench_internal=False):
    """bench_internal: big I/O tensors become device-internal scratch so
    per-call host staging vanishes — used only for device-time measurement."""
    from contextlib import ExitStack

    import concourse.tile as tile
    from concourse import bacc, mybir

    f32 = mybir.dt.float32
    bf = mybir.dt.bfloat16
    AF = mybir.ActivationFunctionType
    ALU = mybir.AluOpType

    nc = bacc.Bacc(
        "TRN2", target_bir_lowering=False, debug=False, num_devices=NCORES
    )
    big = "Internal" if bench_internal else None
    xT = nc.dram_tensor(
        "xT", [C, (S - NBF) * N], f32, kind=big or "ExternalInput"
    ).ap()
    xTb = nc.dram_tensor("xTb", [C, NBF * N], bf, kind=big or "ExternalInput").ap()
    wxb = nc.dram_tensor("wxb", [C, D], bf, kind="ExternalInput").ap()
    wxt = nc.dram_tensor("wxt", [C, D], f32, kind="ExternalInput").ap()
    wht = nc.dram_tensor("wht", [D, D], f32, kind="ExternalInput").ap()
    wyt = nc.dram_tensor("wyt", [D, K], f32, kind="ExternalInput").ap()
    bx = nc.dram_tensor("bx", [D, 1], f32, kind="ExternalInput").ap()
    by = nc.dram_tensor("by", [K, 1], f32, kind="ExternalInput").ap()
    y_o = nc.dram_tensor("y", [K, OWN * N], f32, kind=big or "ExternalOutput").ap()
    h_o = nc.dram_tensor("h", [D, OWN * N], f32, kind=big or "ExternalOutput").ap()
    dummy = None
    if bench_internal:
        dummy = nc.dram_tensor(
            "bench_out", [1, 1], f32, kind="ExternalOutput"
        ).ap()

    PAIRS = S // 2

    with ExitStack() as ctx:
        tc = ctx.enter_context(tile.TileContext(nc))
        consts = ctx.enter_context(tc.tile_pool(name="consts", bufs=1))
        xtp = ctx.enter_context(tc.tile_pool(name="xt", bufs=12))
        gqp = ctx.enter_context(tc.tile_pool(name="gq", bufs=5))
        styp = ctx.enter_context(tc.tile_pool(name="sty", bufs=4))
        recp = ctx.enter_context(tc.tile_pool(name="rec", bufs=3, space="PSUM"))
        yqp = ctx.enter_context(tc.tile_pool(name="yq", bufs=2, space="PSUM"))
        filp = ctx.enter_context(tc.tile_pool(name="fil", bufs=1, space="PSUM"))

        wxt_sb = consts.tile([C, D], f32)
        nc.sync.dma_start(wxt_sb[:], wxt)
        wxb_sb = consts.tile([C, D], bf)
        nc.sync.dma_start(wxb_sb[:], wxb)
        wht_sb = consts.tile([D, D], f32)
        nc.sync.dma_start(wht_sb[:], wht)
        wyt_sb = consts.tile([D, K], f32)
        nc.sync.dma_start(wyt_sb[:], wyt)
        bx_sb = consts.tile([D, 1], f32)
        nc.sync.dma_start(bx_sb[:], bx)
        by_sb = consts.tile([K, 1], f32)
        nc.sync.dma_start(by_sb[:], by)

        # HAM keep-warm filler: a 1-output-row bf16 matmul streaming 256
        # columns keeps the PE array "busy" through the per-step relu
        # windows, so the clock gate stays at 2.4 GHz instead of
        # re-throttling to 1.2 GHz (which doubles every real matmul).
        fill_w = consts.tile([D, 1], bf)
        nc.vector.memset(fill_w[:], 0.0)
        fill_x = consts.tile([D, 2 * N], bf)
        nc.vector.memset(fill_x[:], 0.0)
        fil_ps = filp.tile([1, 2 * N], f32)

        def emit_filler(ncols):
            nc.tensor.matmul(
                fil_ps[0:1, 0:ncols],
                fill_w[:],
                fill_x[:, 0:ncols],
                start=True,
                stop=True,
            )

        def emit_rep():
            rec_tiles = {}
            gq_tiles = {}
            yq_tiles = {}

            def emit_xproj(p):
                if p >= PAIRS:
                    return
                if p < BF_PAIRS:
                    xt_t = xtp.tile([C, 2 * N], bf, name="xtb_t", tag="xtb_t")
                    nc.sync.dma_start(
                        xt_t[:], xTb[:, p * 2 * N : (p + 1) * 2 * N]
                    )
                    lhs = wxb_sb
                else:
                    xt_t = xtp.tile([C, 2 * N], f32, name="xt_t", tag="xt_t")
                    q = p - BF_PAIRS
                    nc.sync.dma_start(
                        xt_t[:], xT[:, q * 2 * N : (q + 1) * 2 * N]
                    )
                    lhs = wxt_sb
                r = recp.tile([D, 2 * N], f32, name="rec_t", tag="rec_t")
                nc.tensor.matmul(r[:], lhs[:], xt_t[:], start=True, stop=True)
                rec_tiles[p] = r

            def emit_y(s, g_sl):
                """Deferred y^T matmul for step s, plus per-quad evac+DMA."""
                if s < BURN:
                    return
                o = s - BURN
                q, e = divmod(o, 4)
                if e == 0:
                    yq_tiles[q] = yqp.tile(
                        [K, 4 * N], f32, name="yq_t", tag="yq_t"
                    )
                yq = yq_tiles[q]
                # has_written clearing is per PSUM bank; the quad tile spans
                # two banks (slices 0-1 and 2-3), so the first slice landing
                # in each bank opens/closes that bank's group and the second
                # overwrites via the cleared has_written bits.
                opener = e % 2 == 0
                nc.tensor.matmul(
                    yq[:, e * N : (e + 1) * N],
                    wyt_sb[:],
                    g_sl,
                    start=opener,
                    stop=opener,
                    skip_group_check=not opener,
                )
                if e == 3:
                    sty = styp.tile([K, 4 * N], f32, name="sty_t", tag="sty_t")
                    # copy + per-partition b_y bias in one ACT op (keeps the
                    # evacuation off the DVE, which carries the B-half relus)
                    nc.scalar.activation(
                        sty[:], yq[:], AF.Identity, bias=by_sb[:]
                    )
                    nc.gpsimd.dma_start(
                        y_o[:, q * 4 * N : (q + 1) * 4 * N], sty[:]
                    )
                    del yq_tiles[q]

            for p in range(PF):
                emit_xproj(p)

            g_prev = None  # (tile, col_base) of previous step's g
            pend = None
            for s in range(S):
                p, e2 = divmod(s, 2)
                quad, e4 = divmod(s, 4)
                rec = rec_tiles[p]
                base = e2 * N
                if s > 0:
                    pt, pb = g_prev
                    nc.tensor.matmul(
                        rec[:, base : base + HALF],
                        wht_sb[:],
                        pt[:, pb : pb + HALF],
                        start=False,
                        stop=False,
                        skip_group_check=True,
                    )
                    nc.tensor.matmul(
                        rec[:, base + HALF : base + N],
                        wht_sb[:],
                        pt[:, pb + HALF : pb + N],
                        start=False,
                        stop=False,
                        skip_group_check=True,
                    )
                if e2 == 0:
                    emit_xproj(p + PF)
                if pend is not None:
                    emit_y(*pend)
                for _f in range(3 if s < BURN else 2):
                    emit_filler(2 * N)
                if e4 == 0:
                    gq_tiles[quad] = gqp.tile(
                        [D, 4 * N], f32, name="gq_t", tag="gq_t"
                    )
                gq = gq_tiles[quad]
                gb = e4 * N
                nc.scalar.activation(
                    gq[:, gb : gb + HALF],
                    rec[:, base : base + HALF],
                    AF.Relu,
                    bias=bx_sb[:],
                )
                nc.vector.tensor_scalar(
                    gq[:, gb + HALF : gb + N],
                    rec[:, base + HALF : base + N],
                    bx_sb[:],
                    0.0,
                    ALU.add,
                    ALU.max,
                )
                pend = (s, gq[:, gb : gb + N])
                g_prev = (gq, gb)
                if e4 == 3 and s >= BURN:
                    oq = quad - BURN // 4
                    nc.gpsimd.dma_start(
                        h_o[:, oq * 4 * N : (oq + 1) * 4 * N], gq[:]
                    )
                if e4 == 3 and quad - 1 in gq_tiles:
                    del gq_tiles[quad - 1]
                if e2 == 1:
                    rec_tiles.pop(p, None)
            emit_y(*pend)

        for _rep in range(repeats):
            emit_rep()

        if dummy is not None:
            nc.sync.dma_start(dummy, bx_sb[0:1, 0:1])

    nc.compile()
    return nc


def _get_program(repeats=1, bench_internal=False):
    key = (repeats, bench_internal)
    if key not in _prog_cache:
        _prog_cache[key] = _build_program(repeats, bench_internal)
    return _prog_cache[key]


def _prep_inputs(x, W_x, b_x, W_h, W_y, b_y):
    x = np.ascontiguousarray(x, np.float32)
    W_x = np.asarray(W_x, np.float32)
    b_x = np.asarray(b_x, np.float32)
    W_h = np.asarray(W_h, np.float32)
    W_y = np.asarray(W_y, np.float32)
    b_y = np.asarray(b_y, np.float32)

    # core-0 burn-in forcing vector: W_x @ x_star = -FORCE (relu clamps to 0)
    lam = np.linalg.solve(
        W_x.astype(np.float64) @ W_x.astype(np.float64).T,
        -FORCE * np.ones(D, np.float64),
    )
    x_star = (W_x.astype(np.float64).T @ lam).astype(np.float32)

    wxt = np.ascontiguousarray(W_x.T)                  # (C, D)
    wht = np.ascontiguousarray(W_h.T)                  # (D, D)
    wyt = np.ascontiguousarray(W_y.T)                  # (D, K)
    bxc = np.ascontiguousarray(b_x[:, None])           # (D, 1)
    byc = np.ascontiguousarray(b_y[:, None])           # (K, 1)

    import ml_dtypes

    wxb = W_x.T.astype(ml_dtypes.bfloat16)

    in_maps = []
    for core in range(NCORES):
        t0 = core * OWN - BURN
        xw = np.empty((S, N, C), np.float32)
        lo = max(0, -t0)  # steps with t < 0 (core 0 only)
        if lo:
            xw[:lo] = x_star[None, None, :]
        xw[lo:] = x[t0 + lo : t0 + S]
        xwT = xw.transpose(2, 0, 1)  # (C, S, N)
        xTb = np.ascontiguousarray(
            xwT[:, :NBF].reshape(C, NBF * N).astype(ml_dtypes.bfloat16)
        )
        xT = np.ascontiguousarray(xwT[:, NBF:].reshape(C, (S - NBF) * N))
        in_maps.append(
            {
                "xT": xT,
                "xTb": xTb,
                "wxb": wxb,
                "wxt": wxt,
                "wht": wht,
                "wyt": wyt,
                "bx": bxc,
                "by": byc,
            }
        )
    return in_maps


def _assemble(results):
    """Untranspose per-core (K, OWN*N) / (D, OWN*N) outputs into full
    (T, N, K) / (T, N, D) arrays."""
    y_full = np.empty((T, N, K), np.float32)
    h_full = np.empty((T, N, D), np.float32)
    for i in range(NCORES):
        sl = slice(i * OWN, (i + 1) * OWN)
        y_full[sl] = (
            results[i]["y"].reshape(K, OWN, N).transpose(1, 2, 0)
        )
        h_full[sl] = (
            results[i]["h"].reshape(D, OWN, N).transpose(1, 2, 0)
        )
    return y_full, h_full


def _run(in_maps, trace=False, repeats=1):
    from concourse.bass_utils import run_bass_kernel_spmd

    nc = _get_program(repeats)
    return run_bass_kernel_spmd(
        nc, in_maps, list(range(NCORES)), trace=trace
    )


def kernel(x, W_x, b_x, W_h, W_y, b_y):
    in_maps = _prep_inputs(x, W_x, b_x, W_h, W_y, b_y)
    res = _run(in_maps)
    return _assemble(res.results)



# revision 2
# speedup vs baseline: 1.8472x; 1.8472x over previous
"""Elman RNN on 8 Trainium2 NeuronCores.

Strategy: time-shard T=512 across the 8 cores (64 owned steps each) and
exploit the contractivity of the relu recurrence: each core re-runs a
16-step burn-in from h=0 before its owned window, which converges to the
bf16 noise floor (~5e-3 relative error, vs the 2e-2 gate). Core 0 has no
real predecessor steps; its burn-in input is a forcing vector x* with
W_x @ x* = -1e4, so relu clamps h to exactly 0 until its window starts.

Everything on the PE runs in bf16 (1 cycle/col vs 2 for fp32 on trn2),
accumulating in fp32 PSUM. The hidden state is kept in bf16 in SBUF, and
both outputs stream out as bf16 (host upcasts to fp32) — this halves
both PE time and HBM traffic vs the fp32 baseline.

On-chip layout is transposed: the hidden state g = h^T lives as
(D=128 partitions, N=256 free). Per step:
  PE:   psum[:, step] += W_h^T.T @ g_prev      (xproj pre-filled per pair)
  ACT:  gA = relu(psum[:, nA] + b_x)           (batch half A)
  DVE:  gB = relu(psum[:, nB] + b_x)           (batch half B)
Owned steps: y^T = W_y^T.T @ g into PSUM (evacuated per 4-step quad on
ACT with b_y added as a per-partition bias), h^T DMA'd straight from the
g tiles. Both outputs are written transposed — (K, OWN*N) / (D, OWN*N) —
and the host untransposes during reassembly. A narrow keep-warm filler
matmul per step keeps the PE clock from re-throttling during the relu
windows.
"""

import sys

if "/opt/trn_rl_repo" not in sys.path:
    sys.path.insert(0, "/opt/trn_rl_repo")

import numpy as np

T, N, C, D, K = 512, 256, 128, 128, 128
NCORES = 8
OWN = T // NCORES          # 64 owned timesteps per core
BURN = 16                  # burn-in steps (contraction reaches bf16 floor)
S = OWN + BURN             # 80 recurrence steps per core
FORCE = 1.0e4
HALF = N // 2              # 128: batch half per relu chain
PF = 2                     # xproj prefetch depth, in pairs
OQ = OWN // 4              # owned quads (4-step output groups)
FILW = 512                 # filler width (cols)
NFIL = 1                   # fillers per step

_prog_cache = {}


def _build_program(repeats=1, bench_internal=False):
    """bench_internal: big I/O tensors become device-internal scratch so
    per-call host staging vanishes — used only for device-time measurement."""
    from contextlib import ExitStack

    import concourse.tile as tile
    from concourse import bacc, mybir

    f32 = mybir.dt.float32
    bf = mybir.dt.bfloat16
    AF = mybir.ActivationFunctionType
    ALU = mybir.AluOpType

    nc = bacc.Bacc(
        "TRN2", target_bir_lowering=False, debug=False, num_devices=NCORES
    )
    big = "Internal" if bench_internal else None
    xTb = nc.dram_tensor("xTb", [C, S * N], bf, kind=big or "ExternalInput").ap()
    wxb = nc.dram_tensor("wxb", [C, D], bf, kind="ExternalInput").ap()
    wht = nc.dram_tensor("wht", [D, D], bf, kind="ExternalInput").ap()
    wyt = nc.dram_tensor("wyt", [D, K], bf, kind="ExternalInput").ap()
    bx = nc.dram_tensor("bx", [D, 1], f32, kind="ExternalInput").ap()
    by = nc.dram_tensor("by", [K, 1], f32, kind="ExternalInput").ap()
    y_o = nc.dram_tensor("y", [K, OWN * N], bf, kind=big or "ExternalOutput").ap()
    h_o = nc.dram_tensor("h", [D, OWN * N], bf, kind=big or "ExternalOutput").ap()
    dummy = None
    if bench_internal:
        dummy = nc.dram_tensor(
            "bench_out", [1, 1], f32, kind="ExternalOutput"
        ).ap()

    PAIRS = S // 2

    with ExitStack() as ctx:
        tc = ctx.enter_context(tile.TileContext(nc))
        consts = ctx.enter_context(tc.tile_pool(name="consts", bufs=1))
        xtp = ctx.enter_context(tc.tile_pool(name="xt", bufs=12))
        gqp = ctx.enter_context(tc.tile_pool(name="gq", bufs=5))
        styp = ctx.enter_context(tc.tile_pool(name="sty", bufs=4))
        recp = ctx.enter_context(tc.tile_pool(name="rec", bufs=3, space="PSUM"))
        yqp = ctx.enter_context(tc.tile_pool(name="yq", bufs=2, space="PSUM"))
        filp = ctx.enter_context(tc.tile_pool(name="fil", bufs=1, space="PSUM"))

        wxb_sb = consts.tile([C, D], bf)
        nc.sync.dma_start(wxb_sb[:], wxb)
        wht_sb = consts.tile([D, D], bf)
        nc.sync.dma_start(wht_sb[:], wht)
        wyt_sb = consts.tile([D, K], bf)
        nc.sync.dma_start(wyt_sb[:], wyt)
        bx_sb = consts.tile([D, 1], f32)
        nc.sync.dma_start(bx_sb[:], bx)
        by_sb = consts.tile([K, 1], f32)
        nc.sync.dma_start(by_sb[:], by)

        # HAM keep-warm filler: a 1-output-row bf16 matmul streaming columns
        # keeps the PE array "busy" through the per-step relu windows, so
        # the clock gate stays at 2.4 GHz instead of re-throttling to
        # 1.2 GHz (which doubles every real matmul).
        fill_w = consts.tile([D, 1], bf)
        nc.vector.memset(fill_w[:], 0.0)
        fill_x = consts.tile([D, FILW], bf)
        nc.vector.memset(fill_x[:], 0.0)
        fil_ps = filp.tile([1, FILW], f32)

        def emit_filler(ncols):
            nc.tensor.matmul(
                fil_ps[0:1, 0:ncols],
                fill_w[:],
                fill_x[:, 0:ncols],
                start=True,
                stop=True,
            )

        def emit_rep():
            rec_tiles = {}
            gq_tiles = {}
            yq_tiles = {}

            def emit_xproj(p):
                if p >= PAIRS:
                    return
                xt_t = xtp.tile([C, 2 * N], bf, name="xt_t", tag="xt_t")
                nc.sync.dma_start(xt_t[:], xTb[:, p * 2 * N : (p + 1) * 2 * N])
                r = recp.tile([D, 2 * N], f32, name="rec_t", tag="rec_t")
                nc.tensor.matmul(r[:], wxb_sb[:], xt_t[:], start=True, stop=True)
                rec_tiles[p] = r

            def emit_y(s, g_sl):
                """Deferred y^T matmul for step s, plus per-quad evac+DMA."""
                if s < BURN:
                    return
                o = s - BURN
                q, e = divmod(o, 4)
                if e == 0:
                    yq_tiles[q] = yqp.tile(
                        [K, 4 * N], f32, name="yq_t", tag="yq_t"
                    )
                yq = yq_tiles[q]
                # has_written clearing is per PSUM bank; the quad tile spans
                # two banks (slices 0-1 and 2-3), so the first slice landing
                # in each bank opens/closes that bank's group and the second
                # overwrites via the cleared has_written bits.
                opener = e % 2 == 0
                nc.tensor.matmul(
                    yq[:, e * N : (e + 1) * N],
                    wyt_sb[:],
                    g_sl,
                    start=opener,
                    stop=opener,
                    skip_group_check=not opener,
                )
                if e == 3:
                    sty = styp.tile([K, 4 * N], bf, name="sty_t", tag="sty_t")
                    # copy + per-partition b_y bias in one ACT op (keeps the
                    # evacuation off the DVE, which carries the B-half relus)
                    nc.scalar.activation(
                        sty[:], yq[:], AF.Identity, bias=by_sb[:]
                    )
                    nc.gpsimd.dma_start(
                        y_o[:, q * 4 * N : (q + 1) * 4 * N], sty[:]
                    )
                    del yq_tiles[q]

            for p in range(PF):
                emit_xproj(p)

            g_prev = None  # (tile, col_base) of previous step's g
            pend = None
            for s in range(S):
                p, e2 = divmod(s, 2)
                quad, e4 = divmod(s, 4)
                rec = rec_tiles[p]
                base = e2 * N
                if s > 0:
                    pt, pb = g_prev
                    nc.tensor.matmul(
                        rec[:, base : base + HALF],
                        wht_sb[:],
                        pt[:, pb : pb + HALF],
                        start=False,
                        stop=False,
                        skip_group_check=True,
                    )
                    nc.tensor.matmul(
                        rec[:, base + HALF : base + N],
                        wht_sb[:],
                        pt[:, pb + HALF : pb + N],
                        start=False,
                        stop=False,
                        skip_group_check=True,
                    )
                if e2 == 0:
                    emit_xproj(p + PF)
                if pend is not None:
                    emit_y(*pend)
                for _f in range(NFIL):
                    emit_filler(FILW)
                if e4 == 0:
                    gq_tiles[quad] = gqp.tile(
                        [D, 4 * N], bf, name="gq_t", tag="gq_t"
                    )
                gq = gq_tiles[quad]
                gb = e4 * N
                nc.scalar.activation(
                    gq[:, gb : gb + HALF],
                    rec[:, base : base + HALF],
                    AF.Relu,
                    bias=bx_sb[:],
                )
                nc.vector.tensor_scalar(
                    gq[:, gb + HALF : gb + N],
                    rec[:, base + HALF : base + N],
                    bx_sb[:],
                    0.0,
                    ALU.add,
                    ALU.max,
                )
                pend = (s, gq[:, gb : gb + N])
                g_prev = (gq, gb)
                if e4 == 3 and s >= BURN:
                    oq = quad - BURN // 4
                    nc.gpsimd.dma_start(
                        h_o[:, oq * 4 * N : (oq + 1) * 4 * N], gq[:]
                    )
                if e4 == 3 and quad - 1 in gq_tiles:
                    del gq_tiles[quad - 1]
                if e2 == 1:
                    rec_tiles.pop(p, None)
            emit_y(*pend)

        for _rep in range(repeats):
            emit_rep()

        if dummy is not None:
            nc.sync.dma_start(dummy, bx_sb[0:1, 0:1])

    nc.compile()
    return nc


def _get_program(repeats=1, bench_internal=False):
    key = (repeats, bench_internal)
    if key not in _prog_cache:
        _prog_cache[key] = _build_program(repeats, bench_internal)
    return _prog_cache[key]


def _prep_inputs(x, W_x, b_x, W_h, W_y, b_y):
    x = np.ascontiguousarray(x, np.float32)
    W_x = np.asarray(W_x, np.float32)
    b_x = np.asarray(b_x, np.float32)
    W_h = np.asarray(W_h, np.float32)
    W_y = np.asarray(W_y, np.float32)
    b_y = np.asarray(b_y, np.float32)

    # core-0 burn-in forcing vector: W_x @ x_star = -FORCE (relu clamps to 0)
    lam = np.linalg.solve(
        W_x.astype(np.float64) @ W_x.astype(np.float64).T,
        -FORCE * np.ones(D, np.float64),
    )
    x_star = (W_x.astype(np.float64).T @ lam).astype(np.float32)

    import ml_dtypes

    bf16 = ml_dtypes.bfloat16
    wxb = np.ascontiguousarray(W_x.T.astype(bf16))    # (C, D)
    wht = np.ascontiguousarray(W_h.T.astype(bf16))    # (D, D)
    wyt = np.ascontiguousarray(W_y.T.astype(bf16))    # (D, K)
    bxc = np.ascontiguousarray(b_x[:, None])          # (D, 1)
    byc = np.ascontiguousarray(b_y[:, None])          # (K, 1)

    in_maps = []
    for core in range(NCORES):
        t0 = core * OWN - BURN
        xw = np.empty((S, N, C), np.float32)
        lo = max(0, -t0)  # steps with t < 0 (core 0 only)
        if lo:
            xw[:lo] = x_star[None, None, :]
        xw[lo:] = x[t0 + lo : t0 + S]
        xwT = xw.transpose(2, 0, 1)  # (C, S, N)
        xTb = np.ascontiguousarray(xwT.reshape(C, S * N).astype(bf16))
        in_maps.append(
            {
                "xTb": xTb,
                "wxb": wxb,
                "wht": wht,
                "wyt": wyt,
                "bx": bxc,
                "by": byc,
            }
        )
    return in_maps


def _assemble(results):
    """Untranspose per-core (K, OWN*N) / (D, OWN*N) bf16 outputs into full
    fp32 (T, N, K) / (T, N, D) arrays."""
    y_full = np.empty((T, N, K), np.float32)
    h_full = np.empty((T, N, D), np.float32)
    for i in range(NCORES):
        sl = slice(i * OWN, (i + 1) * OWN)
        y_full[sl] = (
            np.asarray(results[i]["y"])
            .astype(np.float32)
            .reshape(K, OWN, N)
            .transpose(1, 2, 0)
        )
        h_full[sl] = (
            np.asarray(results[i]["h"])
            .astype(np.float32)
            .reshape(D, OWN, N)
            .transpose(1, 2, 0)
        )
    return y_full, h_full


def _run(in_maps, trace=False, repeats=1):
    from concourse.bass_utils import run_bass_kernel_spmd

    nc = _get_program(repeats)
    return run_bass_kernel_spmd(
        nc, in_maps, list(range(NCORES)), trace=trace
    )


def kernel(x, W_x, b_x, W_h, W_y, b_y):
    in_maps = _prep_inputs(x, W_x, b_x, W_h, W_y, b_y)
    res = _run(in_maps)
    return _assemble(res.results)


# revision 9
# speedup vs baseline: 1.8963x; 1.0266x over previous
"""Elman RNN on 8 Trainium2 NeuronCores.

Strategy: time-shard T=512 across the 8 cores (64 owned steps each) and
exploit the contractivity of the relu recurrence: each core re-runs a
16-step burn-in from h=0 before its owned window, which converges to the
bf16 noise floor (~5e-3 relative error, vs the 2e-2 gate). Core 0 has no
real predecessor steps; its burn-in input is a forcing vector x* with
W_x @ x* = -1e4, so relu clamps h to exactly 0 until its window starts.

Everything on the PE runs in bf16 (1 cycle/col vs 2 for fp32 on trn2),
accumulating in fp32 PSUM. The hidden state is kept in bf16 in SBUF, and
both outputs stream out as bf16 (host upcasts to fp32) — this halves
both PE time and HBM traffic vs the fp32 baseline.

On-chip layout is transposed: the hidden state g = h^T lives as
(D=128 partitions, N=256 free). Per step:
  PE:   psum[:, step] += W_h^T.T @ g_prev      (xproj pre-filled per pair)
  ACT:  gA = relu(psum[:, nA] + b_x)           (batch half A)
  DVE:  gB = relu(psum[:, nB] + b_x)           (batch half B)
Owned steps: y^T = W_y^T.T @ g into PSUM (evacuated per 4-step quad on
ACT with b_y added as a per-partition bias), h^T DMA'd straight from the
g tiles. Both outputs are written transposed — (K, OWN*N) / (D, OWN*N) —
and the host untransposes during reassembly. A narrow keep-warm filler
matmul per step keeps the PE clock from re-throttling during the relu
windows.
"""

import sys

if "/opt/trn_rl_repo" not in sys.path:
    sys.path.insert(0, "/opt/trn_rl_repo")

import numpy as np

T, N, C, D, K = 512, 256, 128, 128, 128
NCORES = 8
OWN = T // NCORES          # 64 owned timesteps per core
BURN = 16                  # burn-in steps (contraction reaches bf16 floor)
S = OWN + BURN             # 80 recurrence steps per core
FORCE = 1.0e4
HALF = N // 2              # 128: batch half per relu chain
PF = 2                     # xproj prefetch depth, in pairs
OQ = OWN // 4              # owned quads (4-step output groups)
FILW = 256                 # filler width (cols)
NFIL = 2                   # fillers per step

_prog_cache = {}


def _build_program(repeats=1, bench_internal=False):
    """bench_internal: big I/O tensors become device-internal scratch so
    per-call host staging vanishes — used only for device-time measurement."""
    from contextlib import ExitStack

    import concourse.tile as tile
    from concourse import bacc, mybir

    f32 = mybir.dt.float32
    bf = mybir.dt.bfloat16
    AF = mybir.ActivationFunctionType
    ALU = mybir.AluOpType

    nc = bacc.Bacc(
        "TRN2", target_bir_lowering=False, debug=False, num_devices=NCORES
    )
    big = "Internal" if bench_internal else None
    xTb = nc.dram_tensor("xTb", [C, S * N], bf, kind=big or "ExternalInput").ap()
    wxb = nc.dram_tensor("wxb", [C, D], bf, kind="ExternalInput").ap()
    wht = nc.dram_tensor("wht", [D, D], bf, kind="ExternalInput").ap()
    wyt = nc.dram_tensor("wyt", [D, K], bf, kind="ExternalInput").ap()
    bx = nc.dram_tensor("bx", [D, 1], f32, kind="ExternalInput").ap()
    by = nc.dram_tensor("by", [K, 1], f32, kind="ExternalInput").ap()
    y_o = nc.dram_tensor("y", [K, OWN * N], bf, kind=big or "ExternalOutput").ap()
    h_o = nc.dram_tensor("h", [D, OWN * N], bf, kind=big or "ExternalOutput").ap()
    dummy = None
    if bench_internal:
        dummy = nc.dram_tensor(
            "bench_out", [1, 1], f32, kind="ExternalOutput"
        ).ap()

    PAIRS = S // 2

    with ExitStack() as ctx:
        tc = ctx.enter_context(tile.TileContext(nc))
        consts = ctx.enter_context(tc.tile_pool(name="consts", bufs=1))
        xtp = ctx.enter_context(tc.tile_pool(name="xt", bufs=12))
        gqp = ctx.enter_context(tc.tile_pool(name="gq", bufs=5))
        styp = ctx.enter_context(tc.tile_pool(name="sty", bufs=4))
        recp = ctx.enter_context(tc.tile_pool(name="rec", bufs=3, space="PSUM"))
        yqp = ctx.enter_context(tc.tile_pool(name="yq", bufs=2, space="PSUM"))
        filp = ctx.enter_context(tc.tile_pool(name="fil", bufs=1, space="PSUM"))

        wxb_sb = consts.tile([C, D], bf)
        nc.sync.dma_start(wxb_sb[:], wxb)
        wht_sb = consts.tile([D, D], bf)
        nc.sync.dma_start(wht_sb[:], wht)
        wyt_sb = consts.tile([D, K], bf)
        nc.sync.dma_start(wyt_sb[:], wyt)
        bx_sb = consts.tile([D, 1], f32)
        nc.sync.dma_start(bx_sb[:], bx)
        by_sb = consts.tile([K, 1], f32)
        nc.sync.dma_start(by_sb[:], by)

        # HAM keep-warm filler: a 1-output-row bf16 matmul streaming columns
        # keeps the PE array "busy" through the per-step relu windows, so
        # the clock gate stays at 2.4 GHz instead of re-throttling to
        # 1.2 GHz (which doubles every real matmul).
        fill_w = consts.tile([D, 1], bf)
        nc.vector.memset(fill_w[:], 0.0)
        fill_x = consts.tile([D, FILW], bf)
        nc.vector.memset(fill_x[:], 0.0)
        fil_ps = filp.tile([1, FILW], f32)

        def emit_filler(ncols):
            nc.tensor.matmul(
                fil_ps[0:1, 0:ncols],
                fill_w[:],
                fill_x[:, 0:ncols],
                start=True,
                stop=True,
            )

        def emit_rep():
            rec_tiles = {}
            gq_tiles = {}
            yq_tiles = {}

            def emit_xproj(p):
                if p >= PAIRS:
                    return
                xt_t = xtp.tile([C, 2 * N], bf, name="xt_t", tag="xt_t")
                nc.sync.dma_start(xt_t[:], xTb[:, p * 2 * N : (p + 1) * 2 * N])
                r = recp.tile([D, 2 * N], f32, name="rec_t", tag="rec_t")
                nc.tensor.matmul(r[:], wxb_sb[:], xt_t[:], start=True, stop=True)
                rec_tiles[p] = r

            sty_tiles = {}
            evac_pend = []  # (quad, slice_idx 0..7) 128-col evac slices

            def emit_y(s, g_sl):
                """Deferred y^T matmul for step s into the quad PSUM tile."""
                if s < BURN:
                    return
                o = s - BURN
                q, e = divmod(o, 4)
                if e == 0:
                    yq_tiles[q] = yqp.tile(
                        [K, 4 * N], f32, name="yq_t", tag="yq_t"
                    )
                yq = yq_tiles[q]
                # has_written clearing is per PSUM bank; the quad tile spans
                # two banks (slices 0-1 and 2-3), so the first slice landing
                # in each bank opens/closes that bank's group and the second
                # overwrites via the cleared has_written bits.
                opener = e % 2 == 0
                nc.tensor.matmul(
                    yq[:, e * N : (e + 1) * N],
                    wyt_sb[:],
                    g_sl,
                    start=opener,
                    stop=opener,
                    skip_group_check=not opener,
                )
                if e == 3:
                    sty_tiles[q] = styp.tile(
                        [K, 4 * N], bf, name="sty_t", tag="sty_t"
                    )
                    evac_pend.extend((q, i) for i in range(8))

            def emit_evac_slices():
                """Drain up to one 128-col y-evac slice per relu engine.
                Interleaving fine slices with the relus keeps the per-step
                insertion into each engine's queue small, so the recurrence
                chain is never stalled behind a bulk evacuation."""
                for k in range(min(2, len(evac_pend))):
                    q, i = evac_pend.pop(0)
                    yq = yq_tiles[q]
                    sty = sty_tiles[q]
                    sl = slice(i * HALF, (i + 1) * HALF)
                    if k == 0:
                        nc.scalar.activation(
                            sty[:, sl], yq[:, sl], AF.Identity, bias=by_sb[:]
                        )
                    else:
                        nc.vector.tensor_scalar_add(
                            sty[:, sl], yq[:, sl], by_sb[:]
                        )
                    if i == 7:
                        nc.gpsimd.dma_start(
                            y_o[:, q * 4 * N : (q + 1) * 4 * N], sty[:]
                        )
                        del yq_tiles[q]
                        del sty_tiles[q]

            for p in range(PF):
                emit_xproj(p)

            g_prev = None  # (tile, col_base) of previous step's g
            pend = None
            for s in range(S):
                p, e2 = divmod(s, 2)
                quad, e4 = divmod(s, 4)
                rec = rec_tiles[p]
                base = e2 * N
                if s > 0:
                    pt, pb = g_prev
                    nc.tensor.matmul(
                        rec[:, base : base + HALF],
                        wht_sb[:],
                        pt[:, pb : pb + HALF],
                        start=False,
                        stop=False,
                        skip_group_check=True,
                    )
                    nc.tensor.matmul(
                        rec[:, base + HALF : base + N],
                        wht_sb[:],
                        pt[:, pb + HALF : pb + N],
                        start=False,
                        stop=False,
                        skip_group_check=True,
                    )
                if e2 == 0:
                    emit_xproj(p + PF)
                if pend is not None:
                    emit_y(*pend)
                for _f in range(NFIL):
                    emit_filler(FILW)
                if e4 == 0:
                    gq_tiles[quad] = gqp.tile(
                        [D, 4 * N], bf, name="gq_t", tag="gq_t"
                    )
                gq = gq_tiles[quad]
                gb = e4 * N
                nc.scalar.activation(
                    gq[:, gb : gb + HALF],
                    rec[:, base : base + HALF],
                    AF.Relu,
                    bias=bx_sb[:],
                )
                nc.vector.tensor_scalar(
                    gq[:, gb + HALF : gb + N],
                    rec[:, base + HALF : base + N],
                    bx_sb[:],
                    0.0,
                    ALU.add,
                    ALU.max,
                )
                emit_evac_slices()
                pend = (s, gq[:, gb : gb + N])
                g_prev = (gq, gb)
                if e4 == 3 and s >= BURN:
                    oq = quad - BURN // 4
                    nc.sync.dma_start(
                        h_o[:, oq * 4 * N : (oq + 1) * 4 * N], gq[:]
                    )
                if e4 == 3 and quad - 1 in gq_tiles:
                    del gq_tiles[quad - 1]
                if e2 == 1:
                    rec_tiles.pop(p, None)
            emit_y(*pend)
            while evac_pend:
                emit_evac_slices()

        for _rep in range(repeats):
            emit_rep()

        if dummy is not None:
            nc.sync.dma_start(dummy, bx_sb[0:1, 0:1])

    nc.compile()
    return nc


def _get_program(repeats=1, bench_internal=False):
    key = (repeats, bench_internal)
    if key not in _prog_cache:
        _prog_cache[key] = _build_program(repeats, bench_internal)
    return _prog_cache[key]


def _prep_inputs(x, W_x, b_x, W_h, W_y, b_y):
    x = np.ascontiguousarray(x, np.float32)
    W_x = np.asarray(W_x, np.float32)
    b_x = np.asarray(b_x, np.float32)
    W_h = np.asarray(W_h, np.float32)
    W_y = np.asarray(W_y, np.float32)
    b_y = np.asarray(b_y, np.float32)

    # core-0 burn-in forcing vector: W_x @ x_star = -FORCE (relu clamps to 0)
    lam = np.linalg.solve(
        W_x.astype(np.float64) @ W_x.astype(np.float64).T,
        -FORCE * np.ones(D, np.float64),
    )
    x_star = (W_x.astype(np.float64).T @ lam).astype(np.float32)

    import ml_dtypes

    bf16 = ml_dtypes.bfloat16
    wxb = np.ascontiguousarray(W_x.T.astype(bf16))    # (C, D)
    wht = np.ascontiguousarray(W_h.T.astype(bf16))    # (D, D)
    wyt = np.ascontiguousarray(W_y.T.astype(bf16))    # (D, K)
    bxc = np.ascontiguousarray(b_x[:, None])          # (D, 1)
    byc = np.ascontiguousarray(b_y[:, None])          # (K, 1)

    in_maps = []
    for core in range(NCORES):
        t0 = core * OWN - BURN
        xw = np.empty((S, N, C), np.float32)
        lo = max(0, -t0)  # steps with t < 0 (core 0 only)
        if lo:
            xw[:lo] = x_star[None, None, :]
        xw[lo:] = x[t0 + lo : t0 + S]
        xwT = xw.transpose(2, 0, 1)  # (C, S, N)
        xTb = np.ascontiguousarray(xwT.reshape(C, S * N).astype(bf16))
        in_maps.append(
            {
                "xTb": xTb,
                "wxb": wxb,
                "wht": wht,
                "wyt": wyt,
                "bx": bxc,
                "by": byc,
            }
        )
    return in_maps


def _assemble(results):
    """Untranspose per-core (K, OWN*N) / (D, OWN*N) bf16 outputs into full
    fp32 (T, N, K) / (T, N, D) arrays."""
    y_full = np.empty((T, N, K), np.float32)
    h_full = np.empty((T, N, D), np.float32)
    for i in range(NCORES):
        sl = slice(i * OWN, (i + 1) * OWN)
        y_full[sl] = (
            np.asarray(results[i]["y"])
            .astype(np.float32)
            .reshape(K, OWN, N)
            .transpose(1, 2, 0)
        )
        h_full[sl] = (
            np.asarray(results[i]["h"])
            .astype(np.float32)
            .reshape(D, OWN, N)
            .transpose(1, 2, 0)
        )
    return y_full, h_full


def _run(in_maps, trace=False, repeats=1):
    from concourse.bass_utils import run_bass_kernel_spmd

    nc = _get_program(repeats)
    return run_bass_kernel_spmd(
        nc, in_maps, list(range(NCORES)), trace=trace
    )


def kernel(x, W_x, b_x, W_h, W_y, b_y):
    in_maps = _prep_inputs(x, W_x, b_x, W_h, W_y, b_y)
    res = _run(in_maps)
    return _assemble(res.results)
